# revision 1
# baseline (speedup 1.0000x reference)
"""GINEConv 4-layer encoder on 8 Trainium2 NeuronCores.

Strategy (graph/data parallel per sharding hint):
  - Nodes partitioned across 8 cores (6250 each, contiguous).  Each core owns
    the edges whose *dst* it owns and computes aggregation + MLP + BN for its
    own nodes only.  Full node features are exchanged via AllGather each layer
    (random graph => halo == everything).
  - Message phase: z table (bf16, node-major) in DRAM; per-edge rows fetched
    with dma_gather(transpose=True) giving feature-major [128, n_edges] tiles;
    msg = relu(gather + edge_attr) on DVE; aggregation = segmented
    tensor_reduce over host-built degree-bucketed, padded edge streams.
  - dma_gather indices are int16 (max 32768 rows), so the table is split in
    two halves (src node < 25000 vs >= 25000) and each dst's edges are
    bucketed per half: cell (B0, B1) = pow2-padded in-degree per half.
  - MLP (two 128x128 matmuls) on TensorE in f32; BatchNorm via local
    sum/sumsq (ACT accum) + tiny AllReduce; normalize+relu fused in one
    ScalarE activation (per-partition scale/bias).
  - SPMD: one program for all cores; all shapes uniform across cores (cell
    counts maxed over cores, dummy dst columns padded; stats corrected by a
    per-core dummy-count input).
"""

import sys

for p in ("/opt/trn_rl_repo",):
    if p not in sys.path:
        sys.path.insert(0, p)

import numpy as np
import ml_dtypes

import concourse.bacc as bacc
import concourse.bass as bass
import concourse.mybir as mybir
import concourse.tile as tile
from concourse.bass_utils import run_bass_kernel_spmd
from concourse.masks import make_identity

# problem constants (hardcoded per harness contract)
N_NODES = 50000
N_EDGES = 640000
H = 128
N_LAYERS = 4
BN_EPS = 1e-5
CORES = 8
NOWN = N_NODES // CORES  # 6250
HALF_NODE = N_NODES // 2  # src < 25000 -> half 0
CH = 8192  # slots per gather call / msg tile
F32 = mybir.dt.float32
BF16 = mybir.dt.bfloat16
I16 = mybir.dt.int16


def _pow2ceil(d):
    d = np.asarray(d)
    out = np.zeros_like(d)
    nz = d > 0
    out[nz] = 1 << np.ceil(np.log2(d[nz])).astype(np.int64)
    return out


class _Desc:
    __slots__ = ("chunk", "off", "nd", "b", "col", "sect")

    def __init__(self, chunk, off, nd, b, col, sect):
        self.chunk, self.off, self.nd, self.b, self.col, self.sect = (
            chunk, off, nd, b, col, sect,
        )


def _build_structure(src, dst):
    """Uniform (cross-core) stream/cell structure + per-core slot data."""
    core_of_dst = dst // NOWN
    half_of_src = (src >= HALF_NODE).astype(np.int64)

    # counts per (dst, half)
    key_dh = dst * 2 + half_of_src
    cnt = np.bincount(key_dh, minlength=2 * N_NODES)
    d0 = cnt[0::2]
    d1 = cnt[1::2]
    B0 = _pow2ceil(d0)
    B1 = _pow2ceil(d1)

    # order edges grouped by (dst, half); within groups order arbitrary
    edge_order = np.argsort(key_dh, kind="stable")
    grp_starts = np.zeros(2 * N_NODES, dtype=np.int64)
    grp_starts[1:] = np.cumsum(cnt)[:-1]

    # per-core cell membership
    cell_key = B0 * 100000 + B1  # scalar key per node
    cells_all = {}
    percore_members = []  # core -> {cellkey: np.array local dst ids}
    for c in range(CORES):
        nodes = np.arange(c * NOWN, (c + 1) * NOWN)
        k = cell_key[nodes]
        order = np.argsort(k, kind="stable")
        ks, idx_start = np.unique(k[order], return_index=True)
        members = {}
        for i, kk in enumerate(ks):
            a = idx_start[i]
            b = idx_start[i + 1] if i + 1 < len(ks) else NOWN
            members[int(kk)] = nodes[order[a:b]]  # global node ids
        percore_members.append(members)
        for kk, m in members.items():
            cells_all[kk] = max(cells_all.get(kk, 0), len(m))

    # pad target width W; guarantee >=1 dummy in cell (0,0) for every core
    w_raw = sum(cells_all.values())
    if 0 not in cells_all:
        cells_all[0] = 0
    W = ((w_raw + 1 + 127) // 128 + 0) * 128
    cells_all[0] += W - w_raw
    assert 16 + 4 * W <= 32768, f"W={W} too large for int16 half-tables"

    cell_keys = sorted(cells_all)  # (0,0) first
    col_start = {}
    pos = 0
    for kk in cell_keys:
        col_start[kk] = pos
        pos += cells_all[kk]
    assert pos == W
    vcol = col_start[0] + cells_all[0] - 1  # guaranteed dummy column

    # column -> node per core; node -> column global
    col2node = np.full((CORES, W), -1, dtype=np.int64)
    for c in range(CORES):
        for kk in cell_keys:
            m = percore_members[c].get(kk)
            if m is not None and len(m):
                s = col_start[kk]
                col2node[c, s : s + len(m)] = m
    node_col = np.zeros(N_NODES, dtype=np.int64)
    for c in range(CORES):
        real = col2node[c] >= 0
        node_col[col2node[c][real]] = np.nonzero(real)[0]

    # stream layout (shared across cores): walk cells per section
    descs = []
    sect_len = []
    for sect in (0, 1):
        pos = 0
        for kk in cell_keys:
            b = (kk // 100000) if sect == 0 else (kk % 100000)
            if b == 0:
                continue
            nd_left = cells_all[kk]
            col = col_start[kk]
            while nd_left:
                room = (-pos) % CH
                if room == 0:
                    room = CH
                fit = min(nd_left, room // b)
                if fit == 0:
                    pos += room  # filler to chunk boundary
                    continue
                descs.append(_Desc(pos // CH, pos % CH, fit, b, col, sect))
                pos += fit * b
                col += fit
                nd_left -= fit
        pos = ((pos + CH - 1) // CH) * CH
        sect_len.append(pos)

    S0, S1 = sect_len
    NCH0 = S0 // CH
    # shift section-1 descriptors to global chunk ids
    for d in descs:
        if d.sect == 1:
            d.chunk += NCH0
    S = S0 + S1
    HALF_ROWS = 16 + 4 * W  # table rows in half 0
    R = 32 + 8 * W

    return dict(
        d0=d0, d1=d1, B0=B0, B1=B1, W=W, S0=S0, S1=S1, S=S, NCH0=NCH0,
        R=R, HALF_ROWS=HALF_ROWS, vcol=vcol, descs=descs,
        col2node=col2node, node_col=node_col,
        edge_order=edge_order, grp_starts=grp_starts, cnt=cnt,
        cells_all=cells_all, cell_keys=cell_keys, col_start=col_start,
    )


def _fill_core_streams(st, src, core):
    """Per-core slot arrays: table row per slot + edge id per slot."""
    W, S, CH_ = st["W"], st["S"], CH
    HALF_ROWS = st["HALF_ROWS"]
    clamp = [0, 16 + 8 * W]  # zero rows: head block (half0), tail block (half1)
    slot_row = np.full(S, -1, dtype=np.int64)
    slot_edge = np.full(S, -1, dtype=np.int64)

    node_pos = 16 + (np.arange(N_NODES) // NOWN) * W + st["node_col"]  # table row

    for d in st["descs"]:
        sect = d.sect
        base = d.chunk * CH_ + d.off
        cols = np.arange(d.col, d.col + d.nd)
        nodes = st["col2node"][core, cols]  # may be -1 (dummy)
        # padded [nd, b] matrices
        rows = np.full((d.nd, d.b), clamp[sect], dtype=np.int64)
        eids = np.full((d.nd, d.b), -1, dtype=np.int64)
        realm = nodes >= 0
        if realm.any():
            rn = nodes[realm]
            g = rn * 2 + sect
            counts = st["cnt"][g]
            starts = st["grp_starts"][g]
            tot = counts.sum()
            if tot:
                # gather the edge ids for all real dsts in this desc
                reps = np.repeat(np.arange(len(rn)), counts)
                offs = np.arange(tot) - np.repeat(
                    np.cumsum(counts) - counts, counts
                )
                eid = st["edge_order"][
                    np.repeat(starts, counts) + offs
                ]
                rrows = np.nonzero(realm)[0][reps]
                rows[rrows, offs] = node_pos[src[eid]]
                eids[rrows, offs] = eid
        sl = slice(base, base + d.nd * d.b)
        slot_row[sl] = rows.reshape(-1)
        slot_edge[sl] = eids.reshape(-1)

    # fillers / unused slots -> clamp row of their section
    for sect, lo, hi in ((0, 0, st["S0"]), (1, st["S0"], S)):
        seg = slot_row[lo:hi]
        seg[seg < 0] = clamp[sect]
    # int16 local indices
    idx = slot_row.copy()
    idx[st["S0"]:] -= HALF_ROWS
    assert idx.min() >= 0 and idx.max() < 32768
    return slot_row, slot_edge, idx.astype(np.int16)


def _build_program(st):
    W, S, R = st["W"], st["S"], st["R"]
    HALF_ROWS, NCH0 = st["HALF_ROWS"], st["NCH0"]
    NCH = S // CH
    NB = W // 128
    vcol = st["vcol"]

    nc = bacc.Bacc(
        "TRN2", target_bir_lowering=False, debug=False, num_devices=CORES
    )

    t_idx = nc.dram_tensor("idx", [128, S // 16], I16, kind="ExternalInput")
    t_ea = nc.dram_tensor("ea", [H, S], F32, kind="ExternalInput")
    t_xnm = nc.dram_tensor("xnm", [W, H], F32, kind="ExternalInput")
    t_xfm = nc.dram_tensor("xfm", [H, W], F32, kind="ExternalInput")
    t_w1 = nc.dram_tensor("w1", [N_LAYERS, H, H], F32, kind="ExternalInput")
    t_w2 = nc.dram_tensor("w2", [N_LAYERS, H, H], F32, kind="ExternalInput")
    t_vecs = nc.dram_tensor("vecs", [H, 32], F32, kind="ExternalInput")
    t_out = nc.dram_tensor("out", [H, W], F32, kind="ExternalOutput")

    # vec column map (matches host packing below)
    VB1, VG1, VBE1, VB2 = 0, 4, 8, 12
    VBNG, VBNB, VNDUM = 16, 19, 22
    # scratch vec columns
    VSUM, VSQ, VASUM, VASQ = 23, 24, 25, 26
    rg = [list(range(CORES))]

    with tile.TileContext(nc) as tc:
        with (
            tc.tile_pool(name="sb", bufs=1) as sb,
            tc.tile_pool(name="db", bufs=2) as db,
            tc.tile_pool(name="ps", bufs=2, space="PSUM") as ps,
            tc.tile_pool(name="pt", bufs=2, space="PSUM") as pt,
            tc.tile_pool(name="dram", bufs=1, space="DRAM") as dr,
        ):
            idx_sb = sb.tile([128, S // 16], I16)
            z = sb.tile([128, W], F32)
            agg = sb.tile([128, W], F32)
            agg2 = sb.tile([128, W], F32)
            wt = sb.tile([128, 8 * H], F32)
            vec = sb.tile([128, 32], F32)
            sc = sb.tile([128, 16], F32)  # scratch per-partition scalars
            ident = sb.tile([128, 128], F32)
            znm = sb.tile([128, NB, 128], BF16)
            sq16 = sb.tile([128, W], BF16)
            zhead = sb.tile([16, H], BF16)

            table = dr.tile([R, H], BF16)
            ea16 = dr.tile([H, S], BF16)
            agi = dr.tile([W, H], BF16)
            sti = dr.tile([128, 2], F32)
            sto = dr.tile([128, 2], F32)

            make_identity(nc, ident[:])

            # ---- one-time init ----
            nc.sync.dma_start(out=idx_sb[:], in_=t_idx[:])
            nc.sync.dma_start(out=z[:], in_=t_xfm[:])
            nc.sync.dma_start(
                out=wt[:, 0 : 4 * H].rearrange("p (l o) -> p l o", l=N_LAYERS),
                in_=t_w1.rearrange("l i o -> i l o"),
            )
            nc.sync.dma_start(
                out=wt[:, 4 * H : 8 * H].rearrange("p (l o) -> p l o", l=N_LAYERS),
                in_=t_w2.rearrange("l i o -> i l o"),
            )
            nc.sync.dma_start(out=vec[:], in_=t_vecs[:])
            # zero table head/tail clamp rows
            nc.vector.memset(zhead[:], 0)
            nc.sync.dma_start(out=table[0:16, :], in_=zhead[:])
            nc.sync.dma_start(out=table[16 + 8 * W : R, :], in_=zhead[:])
            # edge_attr f32 -> bf16 (cast-on-DMA, DRAM->DRAM)
            nc.gpsimd.dma_start(out=ea16[:, :], in_=t_ea[:, :])
            # layer-0 table: cast own x block and AllGather
            nc.gpsimd.dma_start(out=agi[:, :], in_=t_xnm[:, :])

            def allgather():
                nc.gpsimd.collective_compute(
                    "AllGather",
                    mybir.AluOpType.bypass,
                    replica_groups=rg,
                    ins=[agi[:, :]],
                    outs=[table[16 : 16 + 8 * W, :]],
                )

            allgather()

            descs_by_chunk = [[] for _ in range(NCH)]
            for d in st["descs"]:
                descs_by_chunk[d.chunk].append(d)

            def bn_stats_and_coefs(h_t, gcol, bcol, extra_relu):
                """stats(h) -> AllReduce -> write scale into sc[:,0], bias sc[:,1]."""
                nc.scalar.activation(
                    out=sq16[:], in_=h_t[:], func=mybir.ActivationFunctionType.Identity,
                    accum_out=vec[:, VSUM : VSUM + 1],
                )
                nc.scalar.activation(
                    out=sq16[:], in_=h_t[:], func=mybir.ActivationFunctionType.Square,
                    accum_out=vec[:, VSQ : VSQ + 1],
                )
                v = h_t[:, vcol : vcol + 1]
                # sum -= ndum*v ; sq -= ndum*v^2
                nc.vector.tensor_tensor(
                    out=sc[:, 2:3], in0=vec[:, VNDUM : VNDUM + 1], in1=v,
                    op=mybir.AluOpType.mult,
                )
                nc.vector.tensor_tensor(
                    out=vec[:, VSUM : VSUM + 1], in0=vec[:, VSUM : VSUM + 1],
                    in1=sc[:, 2:3], op=mybir.AluOpType.subtract,
                )
                nc.vector.tensor_tensor(
                    out=sc[:, 3:4], in0=v, in1=v, op=mybir.AluOpType.mult
                )
                nc.vector.tensor_tensor(
                    out=sc[:, 3:4], in0=sc[:, 3:4], in1=vec[:, VNDUM : VNDUM + 1],
                    op=mybir.AluOpType.mult,
                )
                nc.vector.tensor_tensor(
                    out=vec[:, VSQ : VSQ + 1], in0=vec[:, VSQ : VSQ + 1],
                    in1=sc[:, 3:4], op=mybir.AluOpType.subtract,
                )
                nc.sync.dma_start(out=sti[:, :], in_=vec[:, VSUM : VSUM + 2])
                nc.gpsimd.collective_compute(
                    "AllReduce", mybir.AluOpType.add, replica_groups=rg,
                    ins=[sti[:, :]], outs=[sto[:, :]],
                )
                nc.sync.dma_start(out=vec[:, VASUM : VASUM + 2], in_=sto[:, :])
                inv_n = 1.0 / float(N_NODES)
                nc.vector.tensor_scalar_mul(
                    out=sc[:, 4:5], in0=vec[:, VASUM : VASUM + 1], scalar1=inv_n
                )  # mean
                nc.vector.tensor_scalar_mul(
                    out=sc[:, 5:6], in0=vec[:, VASQ : VASQ + 1], scalar1=inv_n
                )  # E[x^2]
                nc.vector.tensor_tensor(
                    out=sc[:, 6:7], in0=sc[:, 4:5], in1=sc[:, 4:5],
                    op=mybir.AluOpType.mult,
                )
                nc.vector.tensor_tensor(
                    out=sc[:, 6:7], in0=sc[:, 5:6], in1=sc[:, 6:7],
                    op=mybir.AluOpType.subtract,
                )  # var
                nc.vector.tensor_scalar_add(
                    out=sc[:, 6:7], in0=sc[:, 6:7], scalar1=float(BN_EPS)
                )
                nc.scalar.activation(
                    out=sc[:, 7:8], in_=sc[:, 6:7],
                    func=mybir.ActivationFunctionType.Sqrt,
                )
                nc.vector.reciprocal(out=sc[:, 8:9], in_=sc[:, 7:8])  # rstd
                nc.vector.tensor_tensor(
                    out=sc[:, 0:1], in0=gcol, in1=sc[:, 8:9], op=mybir.AluOpType.mult
                )  # a = gamma * rstd
                nc.vector.tensor_tensor(
                    out=sc[:, 9:10], in0=sc[:, 4:5], in1=sc[:, 0:1],
                    op=mybir.AluOpType.mult,
                )
                nc.vector.tensor_tensor(
                    out=sc[:, 1:2], in0=bcol, in1=sc[:, 9:10],
                    op=mybir.AluOpType.subtract,
                )  # b = beta - mean*a

            # ---- layers ----
            for L in range(N_LAYERS):
                nc.vector.memset(agg[:], 0)
                nc.vector.memset(agg2[:], 0)
                for k in range(NCH):
                    zg = db.tile([128, CH], BF16, tag="zg")
                    eat = db.tile([128, CH], BF16, tag="ea")
                    if k < NCH0:
                        src_ap = table[0:HALF_ROWS, :]
                    else:
                        src_ap = table[HALF_ROWS:R, :]
                    nc.gpsimd.dma_gather(
                        zg[:].rearrange("p (o c) -> p o c", o=1),
                        src_ap,
                        idx_sb[:, k * (CH // 16) : (k + 1) * (CH // 16)],
                        CH, CH, H, transpose=True, single_packet=False,
                    )
                    nc.sync.dma_start(
                        out=eat[:], in_=ea16[:, k * CH : (k + 1) * CH]
                    )
                    nc.vector.tensor_tensor(
                        out=zg[:], in0=zg[:], in1=eat[:],
                        op=mybir.AluOpType.add,
                    )
                    nc.vector.tensor_scalar_max(
                        out=zg[:], in0=zg[:], scalar1=0.0
                    )
                    for d in descs_by_chunk[k]:
                        tgt = agg if d.sect == 0 else agg2
                        nc.vector.tensor_reduce(
                            out=tgt[:, d.col : d.col + d.nd],
                            in_=zg[:, d.off : d.off + d.nd * d.b].rearrange(
                                "p (n b) -> p n b", b=d.b
                            ),
                            axis=mybir.AxisListType.X,
                            op=mybir.AluOpType.add,
                        )
                # h = z + agg + agg2  (into agg)
                nc.vector.tensor_tensor(
                    out=agg[:], in0=z[:], in1=agg[:], op=mybir.AluOpType.add
                )
                nc.vector.tensor_tensor(
                    out=agg[:], in0=agg[:], in1=agg2[:], op=mybir.AluOpType.add
                )
                # h1 = h @ W1[L] + b1[L] -> agg2
                for j in range(0, W, 512):
                    je = min(j + 512, W)
                    pm = ps.tile([128, 512], F32, tag="mm")
                    nc.tensor.matmul(
                        out=pm[:, : je - j],
                        lhsT=wt[:, L * H : (L + 1) * H],
                        rhs=agg[:, j:je],
                        start=True, stop=True,
                    )
                    nc.scalar.activation(
                        out=agg2[:, j:je], in_=pm[:, : je - j],
                        func=mybir.ActivationFunctionType.Identity,
                        bias=vec[:, VB1 + L : VB1 + L + 1],
                    )
                # BN1 + relu
                bn_stats_and_coefs(
                    agg2, vec[:, VG1 + L : VG1 + L + 1], vec[:, VBE1 + L : VBE1 + L + 1],
                    extra_relu=True,
                )
                nc.scalar.activation(
                    out=agg2[:], in_=agg2[:],
                    func=mybir.ActivationFunctionType.Relu,
                    bias=sc[:, 1:2], scale=sc[:, 0:1],
                )
                # z_raw = relu(h1n @ W2[L] + b2[L]) -> agg
                for j in range(0, W, 512):
                    je = min(j + 512, W)
                    pm = ps.tile([128, 512], F32, tag="mm")
                    nc.tensor.matmul(
                        out=pm[:, : je - j],
                        lhsT=wt[:, (4 + L) * H : (5 + L) * H],
                        rhs=agg2[:, j:je],
                        start=True, stop=True,
                    )
                    nc.scalar.activation(
                        out=agg[:, j:je], in_=pm[:, : je - j],
                        func=mybir.ActivationFunctionType.Relu,
                        bias=vec[:, VB2 + L : VB2 + L + 1],
                    )
                if L < N_LAYERS - 1:
                    # inter-layer BN -> new z
                    bn_stats_and_coefs(
                        agg, vec[:, VBNG + L : VBNG + L + 1],
                        vec[:, VBNB + L : VBNB + L + 1], extra_relu=False,
                    )
                    nc.scalar.activation(
                        out=z[:], in_=agg[:],
                        func=mybir.ActivationFunctionType.Identity,
                        bias=sc[:, 1:2], scale=sc[:, 0:1],
                    )
                    # table update: transpose z -> node-major bf16, AG
                    for jb in range(NB):
                        pmt = pt.tile([128, 128], F32, tag="tr")
                        nc.tensor.transpose(
                            out=pmt[:],
                            in_=z[:, jb * 128 : (jb + 1) * 128],
                            identity=ident[:],
                        )
                        nc.scalar.activation(
                            out=znm[:, jb, :], in_=pmt[:],
                            func=mybir.ActivationFunctionType.Copy,
                        )
                    nc.sync.dma_start(
                        out=agi.rearrange("(nb p) f -> p nb f", p=128),
                        in_=znm[:, :, :],
                    )
                    allgather()
                else:
                    nc.sync.dma_start(out=t_out[:, :], in_=agg[:])

    nc.compile()
    return nc


_CACHE = {}


def prepare(x, edge_index, edge_attr, W1, b1, g1, be1, W2, b2, bn_g, bn_b):
    """Host prep + program build; returns (st, nc, in_maps)."""
    x = np.asarray(x, dtype=np.float32)
    edge_index = np.asarray(edge_index, dtype=np.int64)
    edge_attr = np.asarray(edge_attr, dtype=np.float32)
    src = edge_index[0].astype(np.int64)
    dst = edge_index[1].astype(np.int64)

    ck = hash(edge_index.tobytes())
    if ck in _CACHE:
        st, nc, percore = _CACHE[ck]
    else:
        st = _build_structure(src, dst)
        percore = [
            _fill_core_streams(st, src, c) for c in range(CORES)
        ]
        nc = _build_program(st)
        _CACHE[ck] = (st, nc, percore)

    W, S = st["W"], st["S"]
    # weight vec packing [128, 32]
    vecs_base = np.zeros((H, 32), dtype=np.float32)
    vecs_base[:, 0:4] = np.asarray(b1, np.float32).T
    vecs_base[:, 4:8] = np.asarray(g1, np.float32).T
    vecs_base[:, 8:12] = np.asarray(be1, np.float32).T
    vecs_base[:, 12:16] = np.asarray(b2, np.float32).T
    vecs_base[:, 16:19] = np.asarray(bn_g, np.float32).T
    vecs_base[:, 19:22] = np.asarray(bn_b, np.float32).T

    in_maps = []
    for c in range(CORES):
        slot_row, slot_edge, idx16 = percore[c]
        wrapped = np.ascontiguousarray(idx16.reshape(-1, 16).T)  # [16, S/16]
        idx_full = np.ascontiguousarray(np.tile(wrapped, (8, 1)))  # [128, S/16]

        ea_nm = np.zeros((S, H), dtype=np.float32)
        m = slot_edge >= 0
        ea_nm[m] = edge_attr[slot_edge[m]]
        ea_fm = np.ascontiguousarray(ea_nm.T)

        c2n = st["col2node"][c]
        xn = np.zeros((W, H), dtype=np.float32)
        real = c2n >= 0
        xn[real] = x[c2n[real]]
        xf = np.ascontiguousarray(xn.T)

        vecs = vecs_base.copy()
        vecs[:, 22] = float(W - NOWN)  # dummy count

        in_maps.append(
            {
                "idx": idx_full,
                "ea": ea_fm,
                "xnm": xn,
                "xfm": xf,
                "w1": np.ascontiguousarray(np.asarray(W1, np.float32)),
                "w2": np.ascontiguousarray(np.asarray(W2, np.float32)),
                "vecs": vecs,
            }
        )

    return st, nc, in_maps


def unshard(st, results):
    out = np.zeros((N_NODES, H), dtype=np.float32)
    for c in range(CORES):
        o = results[c]["out"]  # [H, W]
        c2n = st["col2node"][c]
        real = c2n >= 0
        out[c2n[real]] = o.T[real]
    return out


def kernel(x, edge_index, edge_attr, W1, b1, g1, be1, W2, b2, bn_g, bn_b):
    st, nc, in_maps = prepare(
        x, edge_index, edge_attr, W1, b1, g1, be1, W2, b2, bn_g, bn_b
    )
    res = run_bass_kernel_spmd(nc, in_maps, core_ids=list(range(CORES)))
    kernel.last_results = res
    return unshard(st, res.results)


if __name__ == "__main__":
    # smoke test with random data
    rng = np.random.default_rng(0)
    x = rng.standard_normal((N_NODES, H)).astype(np.float32)
    ei = rng.integers(0, N_NODES, (2, N_EDGES)).astype(np.int64)
    ea = rng.standard_normal((N_EDGES, H)).astype(np.float32)
    s = 1.0 / np.sqrt(H)
    W1 = (rng.standard_normal((4, H, H)) * s).astype(np.float32)
    W2 = (rng.standard_normal((4, H, H)) * s).astype(np.float32)
    z = np.zeros((4, H), np.float32)
    o = np.ones((4, H), np.float32)
    out = kernel(x, ei, ea, W1, z, o, z, W2, z, np.ones((3, H), np.float32), np.zeros((3, H), np.float32))
    print("out", out.shape, out.dtype, np.abs(out).mean())



# revision 13
# speedup vs baseline: 1.7520x; 1.7520x over previous
"""GINEConv 4-layer encoder on 8 Trainium2 NeuronCores.

Strategy (graph/data parallel per sharding hint):
  - Nodes partitioned across 8 cores (6250 each, contiguous).  Each core owns
    the edges whose *dst* it owns and computes aggregation + MLP + BN for its
    own nodes only.  Full node features are exchanged via AllGather each layer
    (random graph => halo == everything).
  - Message phase: z table (bf16, node-major) in DRAM; per-edge rows fetched
    with dma_gather(transpose=True) giving feature-major [128, n_edges] tiles;
    msg = relu(gather + edge_attr) on DVE; aggregation = segmented
    tensor_reduce over host-built degree-bucketed, padded edge streams.
  - dma_gather indices are int16 (max 32768 rows), so the table is split in
    two halves (src node < 25000 vs >= 25000) and each dst's edges are
    bucketed per half: cell (B0, B1) = pow2-padded in-degree per half.
  - MLP (two 128x128 matmuls) on TensorE in f32; BatchNorm via local
    sum/sumsq (ACT accum) + tiny AllReduce; normalize+relu fused in one
    ScalarE activation (per-partition scale/bias).
  - SPMD: one program for all cores; all shapes uniform across cores (cell
    counts maxed over cores, dummy dst columns padded; stats corrected by a
    per-core dummy-count input).
"""

import sys

for p in ("/opt/trn_rl_repo",):
    if p not in sys.path:
        sys.path.insert(0, p)

import numpy as np
import ml_dtypes

import concourse.bacc as bacc
import concourse.bass as bass
import concourse.mybir as mybir
import concourse.tile as tile
from concourse.bass_utils import run_bass_kernel_spmd
from concourse.masks import make_identity

# problem constants (hardcoded per harness contract)
N_NODES = 50000
N_EDGES = 640000
H = 128
N_LAYERS = 4
BN_EPS = 1e-5
CORES = 8
NOWN = N_NODES // CORES  # 6250
HALF_NODE = N_NODES // 2  # src < 25000 -> half 0
CH = 4096  # slots per gather call / msg tile (must fit SWDGE desc ring)
USE_PREP = False  # prepare_only+trigger pipelining (False = immediate gathers)
F32 = mybir.dt.float32
BF16 = mybir.dt.bfloat16
I16 = mybir.dt.int16


def _pow2ceil(d):
    # v2: exact-degree cells — no pow2 padding. Cell = (d0, d1) pair; the
    # cross-core max still pads dummy columns, but slot count drops ~13%
    # and, more importantly, every slot is a real edge (fewer descriptors
    # for the Q7 SWDGE generator, which is the critical path).
    return np.asarray(d).copy()


class _Desc:
    __slots__ = ("chunk", "off", "nd", "b", "col", "sect")

    def __init__(self, chunk, off, nd, b, col, sect):
        self.chunk, self.off, self.nd, self.b, self.col, self.sect = (
            chunk, off, nd, b, col, sect,
        )


def _build_structure(src, dst):
    """Uniform (cross-core) stream/cell structure + per-core slot data."""
    core_of_dst = dst // NOWN
    half_of_src = (src >= HALF_NODE).astype(np.int64)

    # counts per (dst, half)
    key_dh = dst * 2 + half_of_src
    cnt = np.bincount(key_dh, minlength=2 * N_NODES)
    d0 = cnt[0::2]
    d1 = cnt[1::2]
    B0 = _pow2ceil(d0)
    B1 = _pow2ceil(d1)

    # order edges grouped by (dst, half); within groups order arbitrary
    edge_order = np.argsort(key_dh, kind="stable")
    grp_starts = np.zeros(2 * N_NODES, dtype=np.int64)
    grp_starts[1:] = np.cumsum(cnt)[:-1]

    # per-core cell membership
    cell_key = B0 * 100000 + B1  # scalar key per node
    cells_all = {}
    percore_members = []  # core -> {cellkey: np.array local dst ids}
    for c in range(CORES):
        nodes = np.arange(c * NOWN, (c + 1) * NOWN)
        k = cell_key[nodes]
        order = np.argsort(k, kind="stable")
        ks, idx_start = np.unique(k[order], return_index=True)
        members = {}
        for i, kk in enumerate(ks):
            a = idx_start[i]
            b = idx_start[i + 1] if i + 1 < len(ks) else NOWN
            members[int(kk)] = nodes[order[a:b]]  # global node ids
        percore_members.append(members)
        for kk, m in members.items():
            cells_all[kk] = max(cells_all.get(kk, 0), len(m))

    # pad target width W; guarantee >=1 dummy in cell (0,0) for every core
    w_raw = sum(cells_all.values())
    if 0 not in cells_all:
        cells_all[0] = 0
    W = ((w_raw + 1 + 127) // 128 + 0) * 128
    cells_all[0] += W - w_raw
    assert 16 + 4 * W <= 32768, f"W={W} too large for int16 half-tables"

    cell_keys = sorted(cells_all)  # (0,0) first
    col_start = {}
    pos = 0
    for kk in cell_keys:
        col_start[kk] = pos
        pos += cells_all[kk]
    assert pos == W
    vcol = col_start[0] + cells_all[0] - 1  # guaranteed dummy column

    # column -> node per core; node -> column global
    col2node = np.full((CORES, W), -1, dtype=np.int64)
    for c in range(CORES):
        for kk in cell_keys:
            m = percore_members[c].get(kk)
            if m is not None and len(m):
                s = col_start[kk]
                col2node[c, s : s + len(m)] = m
    node_col = np.zeros(N_NODES, dtype=np.int64)
    for c in range(CORES):
        real = col2node[c] >= 0
        node_col[col2node[c][real]] = np.nonzero(real)[0]

    # stream layout (shared across cores): walk cells per section.
    # v2: section tails round to 128 (gather num_idxs granularity), not CH —
    # no filler slots gathered for the tail chunk.
    descs = []
    sect_len = []
    for sect in (0, 1):
        pos = 0
        for kk in cell_keys:
            b = (kk // 100000) if sect == 0 else (kk % 100000)
            if b == 0:
                continue
            nd_left = cells_all[kk]
            col = col_start[kk]
            while nd_left:
                room = (-pos) % CH
                if room == 0:
                    room = CH
                fit = min(nd_left, room // b)
                if fit == 0:
                    pos += room  # filler to chunk boundary
                    continue
                descs.append(_Desc(pos // CH, pos % CH, fit, b, col, sect))
                pos += fit * b
                col += fit
                nd_left -= fit
        pos = ((pos + 127) // 128) * 128
        sect_len.append(pos)

    S0, S1 = sect_len
    NCH0 = (S0 + CH - 1) // CH
    NCH1 = (S1 + CH - 1) // CH
    # shift section-1 descriptors to global chunk ids
    for d in descs:
        if d.sect == 1:
            d.chunk += NCH0
    S = S0 + S1
    # global chunk table: (start offset in flat stream, size)
    chunk_start, chunk_size = [], []
    for k in range(NCH0):
        chunk_start.append(k * CH)
        chunk_size.append(min(CH, S0 - k * CH))
    for k in range(NCH1):
        chunk_start.append(S0 + k * CH)
        chunk_size.append(min(CH, S1 - k * CH))
    HALF_ROWS = 16 + 4 * W  # table rows in half 0
    R = 32 + 8 * W

    return dict(
        d0=d0, d1=d1, B0=B0, B1=B1, W=W, S0=S0, S1=S1, S=S, NCH0=NCH0,
        R=R, HALF_ROWS=HALF_ROWS, vcol=vcol, descs=descs,
        chunk_start=chunk_start, chunk_size=chunk_size,
        col2node=col2node, node_col=node_col,
        edge_order=edge_order, grp_starts=grp_starts, cnt=cnt,
        cells_all=cells_all, cell_keys=cell_keys, col_start=col_start,
    )


def _fill_core_streams(st, src, core):
    """Per-core slot arrays: table row per slot + edge id per slot."""
    W, S, CH_ = st["W"], st["S"], CH
    HALF_ROWS = st["HALF_ROWS"]
    clamp = [0, 16 + 8 * W]  # zero rows: head block (half0), tail block (half1)
    slot_row = np.full(S, -1, dtype=np.int64)
    slot_edge = np.full(S, -1, dtype=np.int64)

    node_pos = 16 + (np.arange(N_NODES) // NOWN) * W + st["node_col"]  # table row

    for d in st["descs"]:
        sect = d.sect
        base = st["chunk_start"][d.chunk] + d.off
        cols = np.arange(d.col, d.col + d.nd)
        nodes = st["col2node"][core, cols]  # may be -1 (dummy)
        # padded [nd, b] matrices
        rows = np.full((d.nd, d.b), clamp[sect], dtype=np.int64)
        eids = np.full((d.nd, d.b), -1, dtype=np.int64)
        realm = nodes >= 0
        if realm.any():
            rn = nodes[realm]
            g = rn * 2 + sect
            counts = st["cnt"][g]
            starts = st["grp_starts"][g]
            tot = counts.sum()
            if tot:
                # gather the edge ids for all real dsts in this desc
                reps = np.repeat(np.arange(len(rn)), counts)
                offs = np.arange(tot) - np.repeat(
                    np.cumsum(counts) - counts, counts
                )
                eid = st["edge_order"][
                    np.repeat(starts, counts) + offs
                ]
                rrows = np.nonzero(realm)[0][reps]
                rows[rrows, offs] = node_pos[src[eid]]
                eids[rrows, offs] = eid
        sl = slice(base, base + d.nd * d.b)
        slot_row[sl] = rows.reshape(-1)
        slot_edge[sl] = eids.reshape(-1)

    # fillers / unused slots -> clamp row of their section
    for sect, lo, hi in ((0, 0, st["S0"]), (1, st["S0"], S)):
        seg = slot_row[lo:hi]
        seg[seg < 0] = clamp[sect]
    # int16 local indices
    idx = slot_row.copy()
    idx[st["S0"]:] -= HALF_ROWS
    assert idx.min() >= 0 and idx.max() < 32768
    return slot_row, slot_edge, idx.astype(np.int16)


def _build_program(st):
    W, S, R = st["W"], st["S"], st["R"]
    HALF_ROWS, NCH0 = st["HALF_ROWS"], st["NCH0"]
    NCH = len(st["chunk_size"])
    NB = W // 128
    vcol = st["vcol"]
    NQ = 2  # SWDGE queues; alternate so desc-gen(k+1) overlaps drain(k)

    nc = bacc.Bacc(
        "TRN2", target_bir_lowering=False, debug=False, num_devices=CORES,
        num_swdge_queues=NQ,
    )

    t_idx = nc.dram_tensor("idx", [128, S // 16], I16, kind="ExternalInput")
    t_ea = nc.dram_tensor("ea", [H, S], BF16, kind="ExternalInput")
    t_xnm = nc.dram_tensor("xnm", [W, H], BF16, kind="ExternalInput")
    t_xfm = nc.dram_tensor("xfm", [H, W], F32, kind="ExternalInput")
    t_w1 = nc.dram_tensor("w1", [N_LAYERS, H, H], F32, kind="ExternalInput")
    t_w2 = nc.dram_tensor("w2", [N_LAYERS, H, H], F32, kind="ExternalInput")
    t_vecs = nc.dram_tensor("vecs", [H, 32], F32, kind="ExternalInput")
    t_out = nc.dram_tensor("out", [H, W], F32, kind="ExternalOutput")

    # vec column map (matches host packing below)
    VB1, VG1, VBE1, VB2 = 0, 4, 8, 12
    VBNG, VBNB, VNDUM = 16, 19, 22
    # scratch vec columns
    VSUM, VSQ, VASUM, VASQ = 23, 24, 25, 26
    rg = [list(range(CORES))]

    with tile.TileContext(nc) as tc:
        with (
            tc.tile_pool(name="sb", bufs=1) as sb,
            tc.tile_pool(name="db", bufs=2) as db,
            tc.tile_pool(name="ps", bufs=2, space="PSUM") as ps,
            tc.tile_pool(name="pt", bufs=2, space="PSUM") as pt,
            tc.tile_pool(name="dram", bufs=1, space="DRAM") as dr,
        ):
            idx_sb = sb.tile([128, S // 16], I16)
            z = sb.tile([128, W], F32)
            agg = sb.tile([128, W], F32)
            agg2 = sb.tile([128, W], F32)
            wt = sb.tile([128, 8 * H], F32)
            vec = sb.tile([128, 32], F32)
            sc = sb.tile([128, 16], F32)  # scratch per-partition scalars
            ident = sb.tile([128, 128], F32)
            znm = sb.tile([128, NB, 128], BF16)
            sq16 = sb.tile([128, W], BF16)
            zhead = sb.tile([16, H], BF16)

            table = dr.tile([R, H], BF16)
            agi = dr.tile([W, H], BF16)
            sti = dr.tile([128, 2], F32)
            sto = dr.tile([128, 2], F32)

            dma_sems = [nc.alloc_semaphore(f"swdge_dma{q}") for q in range(NQ)]

            make_identity(nc, ident[:])

            # ---- one-time init ----
            nc.sync.dma_start(out=idx_sb[:], in_=t_idx[:])
            nc.sync.dma_start(out=z[:], in_=t_xfm[:])
            nc.sync.dma_start(
                out=wt[:, 0 : 4 * H].rearrange("p (l o) -> p l o", l=N_LAYERS),
                in_=t_w1.rearrange("l i o -> i l o"),
            )
            nc.sync.dma_start(
                out=wt[:, 4 * H : 8 * H].rearrange("p (l o) -> p l o", l=N_LAYERS),
                in_=t_w2.rearrange("l i o -> i l o"),
            )
            nc.sync.dma_start(out=vec[:], in_=t_vecs[:])
            # zero table head/tail clamp rows
            nc.vector.memset(zhead[:], 0)
            nc.sync.dma_start(out=table[0:16, :], in_=zhead[:])
            nc.sync.dma_start(out=table[16 + 8 * W : R, :], in_=zhead[:])
            # layer-0 table: own x block (bf16 from host) and AllGather
            nc.gpsimd.dma_start(out=agi[:, :], in_=t_xnm[:, :])

            def allgather():
                nc.gpsimd.collective_compute(
                    "AllGather",
                    mybir.AluOpType.bypass,
                    replica_groups=rg,
                    ins=[agi[:, :]],
                    outs=[table[16 : 16 + 8 * W, :]],
                )

            allgather()

            descs_by_chunk = [[] for _ in range(NCH)]
            for d in st["descs"]:
                descs_by_chunk[d.chunk].append(d)

            def bn_stats_and_coefs(h_t, gcol, bcol, extra_relu):
                """stats(h) -> AllReduce -> write scale into sc[:,0], bias sc[:,1]."""
                nc.scalar.activation(
                    out=sq16[:], in_=h_t[:], func=mybir.ActivationFunctionType.Identity,
                    accum_out=vec[:, VSUM : VSUM + 1],
                )
                nc.scalar.activation(
                    out=sq16[:], in_=h_t[:], func=mybir.ActivationFunctionType.Square,
                    accum_out=vec[:, VSQ : VSQ + 1],
                )
                v = h_t[:, vcol : vcol + 1]
                # sum -= ndum*v ; sq -= ndum*v^2
                nc.vector.tensor_tensor(
                    out=sc[:, 2:3], in0=vec[:, VNDUM : VNDUM + 1], in1=v,
                    op=mybir.AluOpType.mult,
                )
                nc.vector.tensor_tensor(
                    out=vec[:, VSUM : VSUM + 1], in0=vec[:, VSUM : VSUM + 1],
                    in1=sc[:, 2:3], op=mybir.AluOpType.subtract,
                )
                nc.vector.tensor_tensor(
                    out=sc[:, 3:4], in0=v, in1=v, op=mybir.AluOpType.mult
                )
                nc.vector.tensor_tensor(
                    out=sc[:, 3:4], in0=sc[:, 3:4], in1=vec[:, VNDUM : VNDUM + 1],
                    op=mybir.AluOpType.mult,
                )
                nc.vector.tensor_tensor(
                    out=vec[:, VSQ : VSQ + 1], in0=vec[:, VSQ : VSQ + 1],
                    in1=sc[:, 3:4], op=mybir.AluOpType.subtract,
                )
                nc.sync.dma_start(out=sti[:, :], in_=vec[:, VSUM : VSUM + 2])
                nc.gpsimd.collective_compute(
                    "AllReduce", mybir.AluOpType.add, replica_groups=rg,
                    ins=[sti[:, :]], outs=[sto[:, :]],
                )
                nc.sync.dma_start(out=vec[:, VASUM : VASUM + 2], in_=sto[:, :])
                inv_n = 1.0 / float(N_NODES)
                nc.vector.tensor_scalar_mul(
                    out=sc[:, 4:5], in0=vec[:, VASUM : VASUM + 1], scalar1=inv_n
                )  # mean
                nc.vector.tensor_scalar_mul(
                    out=sc[:, 5:6], in0=vec[:, VASQ : VASQ + 1], scalar1=inv_n
                )  # E[x^2]
                nc.vector.tensor_tensor(
                    out=sc[:, 6:7], in0=sc[:, 4:5], in1=sc[:, 4:5],
                    op=mybir.AluOpType.mult,
                )
                nc.vector.tensor_tensor(
                    out=sc[:, 6:7], in0=sc[:, 5:6], in1=sc[:, 6:7],
                    op=mybir.AluOpType.subtract,
                )  # var
                nc.vector.tensor_scalar_add(
                    out=sc[:, 6:7], in0=sc[:, 6:7], scalar1=float(BN_EPS)
                )
                nc.scalar.activation(
                    out=sc[:, 7:8], in_=sc[:, 6:7],
                    func=mybir.ActivationFunctionType.Sqrt,
                )
                nc.vector.reciprocal(out=sc[:, 8:9], in_=sc[:, 7:8])  # rstd
                nc.vector.tensor_tensor(
                    out=sc[:, 0:1], in0=gcol, in1=sc[:, 8:9], op=mybir.AluOpType.mult
                )  # a = gamma * rstd
                nc.vector.tensor_tensor(
                    out=sc[:, 9:10], in0=sc[:, 4:5], in1=sc[:, 0:1],
                    op=mybir.AluOpType.mult,
                )
                nc.vector.tensor_tensor(
                    out=sc[:, 1:2], in0=bcol, in1=sc[:, 9:10],
                    op=mybir.AluOpType.subtract,
                )  # b = beta - mean*a

            # ---- layers ----
            for L in range(N_LAYERS):
                nc.vector.memset(agg[:], 0)
                nc.vector.memset(agg2[:], 0)
                for k in range(NCH):
                    cs = st["chunk_start"][k]
                    cw = st["chunk_size"][k]
                    zg = db.tile([128, cw], BF16, tag="zg")
                    eat = db.tile([128, cw], BF16, tag="ea")
                    if k < NCH0:
                        src_ap = table[0:HALF_ROWS, :]
                    else:
                        src_ap = table[HALF_ROWS:R, :]
                    q = k % NQ
                    if USE_PREP:
                        # prepare_only: the gather instruction retires at
                        # descriptor-gen completion, so gen(k+1) overlaps
                        # the DMA drain of chunk k instead of serializing.
                        nc.gpsimd.dma_gather(
                            zg[:].rearrange("p (o c) -> p o c", o=1),
                            src_ap,
                            idx_sb[:, cs // 16 : (cs + cw) // 16],
                            cw, cw, H, transpose=True, single_packet=False,
                            prepare_only=True, sem=dma_sems[q], queue_num=q,
                        )
                        nc.gpsimd.trigger_dma(count=None, queue_num=q)
                    else:
                        nc.gpsimd.dma_gather(
                            zg[:].rearrange("p (o c) -> p o c", o=1),
                            src_ap,
                            idx_sb[:, cs // 16 : (cs + cw) // 16],
                            cw, cw, H, transpose=True, single_packet=False,
                        )
                    nc.sync.dma_start(
                        out=eat[:], in_=t_ea[:, cs : cs + cw]
                    )
                    nc.vector.tensor_tensor(
                        out=zg[:], in0=zg[:], in1=eat[:],
                        op=mybir.AluOpType.add,
                    )
                    nc.vector.tensor_scalar_max(
                        out=zg[:], in0=zg[:], scalar1=0.0
                    )
                    for d in descs_by_chunk[k]:
                        tgt = agg if d.sect == 0 else agg2
                        nc.vector.tensor_reduce(
                            out=tgt[:, d.col : d.col + d.nd],
                            in_=zg[:, d.off : d.off + d.nd * d.b].rearrange(
                                "p (n b) -> p n b", b=d.b
                            ),
                            axis=mybir.AxisListType.X,
                            op=mybir.AluOpType.add,
                        )
                # h = z + agg + agg2  (into agg)
                nc.vector.tensor_tensor(
                    out=agg[:], in0=z[:], in1=agg[:], op=mybir.AluOpType.add
                )
                nc.vector.tensor_tensor(
                    out=agg[:], in0=agg[:], in1=agg2[:], op=mybir.AluOpType.add
                )
                # h1 = h @ W1[L] + b1[L] -> agg2
                for j in range(0, W, 512):
                    je = min(j + 512, W)
                    pm = ps.tile([128, 512], F32, tag="mm")
                    nc.tensor.matmul(
                        out=pm[:, : je - j],
                        lhsT=wt[:, L * H : (L + 1) * H],
                        rhs=agg[:, j:je],
                        start=True, stop=True,
                    )
                    nc.scalar.activation(
                        out=agg2[:, j:je], in_=pm[:, : je - j],
                        func=mybir.ActivationFunctionType.Identity,
                        bias=vec[:, VB1 + L : VB1 + L + 1],
                    )
                # BN1 + relu
                bn_stats_and_coefs(
                    agg2, vec[:, VG1 + L : VG1 + L + 1], vec[:, VBE1 + L : VBE1 + L + 1],
                    extra_relu=True,
                )
                nc.scalar.activation(
                    out=agg2[:], in_=agg2[:],
                    func=mybir.ActivationFunctionType.Relu,
                    bias=sc[:, 1:2], scale=sc[:, 0:1],
                )
                # z_raw = relu(h1n @ W2[L] + b2[L]) -> agg
                for j in range(0, W, 512):
                    je = min(j + 512, W)
                    pm = ps.tile([128, 512], F32, tag="mm")
                    nc.tensor.matmul(
                        out=pm[:, : je - j],
                        lhsT=wt[:, (4 + L) * H : (5 + L) * H],
                        rhs=agg2[:, j:je],
                        start=True, stop=True,
                    )
                    nc.scalar.activation(
                        out=agg[:, j:je], in_=pm[:, : je - j],
                        func=mybir.ActivationFunctionType.Relu,
                        bias=vec[:, VB2 + L : VB2 + L + 1],
                    )
                if L < N_LAYERS - 1:
                    # inter-layer BN -> new z
                    bn_stats_and_coefs(
                        agg, vec[:, VBNG + L : VBNG + L + 1],
                        vec[:, VBNB + L : VBNB + L + 1], extra_relu=False,
                    )
                    nc.scalar.activation(
                        out=z[:], in_=agg[:],
                        func=mybir.ActivationFunctionType.Identity,
                        bias=sc[:, 1:2], scale=sc[:, 0:1],
                    )
                    # table update: transpose z -> node-major bf16, AG
                    for jb in range(NB):
                        pmt = pt.tile([128, 128], F32, tag="tr")
                        nc.tensor.transpose(
                            out=pmt[:],
                            in_=z[:, jb * 128 : (jb + 1) * 128],
                            identity=ident[:],
                        )
                        nc.scalar.activation(
                            out=znm[:, jb, :], in_=pmt[:],
                            func=mybir.ActivationFunctionType.Copy,
                        )
                    nc.sync.dma_start(
                        out=agi.rearrange("(nb p) f -> p nb f", p=128),
                        in_=znm[:, :, :],
                    )
                    allgather()
                else:
                    nc.sync.dma_start(out=t_out[:, :], in_=agg[:])

    nc.compile()
    return nc


_CACHE = {}


def prepare(x, edge_index, edge_attr, W1, b1, g1, be1, W2, b2, bn_g, bn_b):
    """Host prep + program build; returns (st, nc, in_maps)."""
    x = np.asarray(x, dtype=np.float32)
    edge_index = np.asarray(edge_index, dtype=np.int64)
    edge_attr = np.asarray(edge_attr, dtype=np.float32)
    src = edge_index[0].astype(np.int64)
    dst = edge_index[1].astype(np.int64)

    ck = hash(edge_index.tobytes())
    if ck in _CACHE:
        st, nc, percore = _CACHE[ck]
    else:
        st = _build_structure(src, dst)
        percore = [
            _fill_core_streams(st, src, c) for c in range(CORES)
        ]
        nc = _build_program(st)
        _CACHE[ck] = (st, nc, percore)

    W, S = st["W"], st["S"]
    # weight vec packing [128, 32]
    vecs_base = np.zeros((H, 32), dtype=np.float32)
    vecs_base[:, 0:4] = np.asarray(b1, np.float32).T
    vecs_base[:, 4:8] = np.asarray(g1, np.float32).T
    vecs_base[:, 8:12] = np.asarray(be1, np.float32).T
    vecs_base[:, 12:16] = np.asarray(b2, np.float32).T
    vecs_base[:, 16:19] = np.asarray(bn_g, np.float32).T
    vecs_base[:, 19:22] = np.asarray(bn_b, np.float32).T

    in_maps = []
    for c in range(CORES):
        slot_row, slot_edge, idx16 = percore[c]
        wrapped = np.ascontiguousarray(idx16.reshape(-1, 16).T)  # [16, S/16]
        idx_full = np.ascontiguousarray(np.tile(wrapped, (8, 1)))  # [128, S/16]

        ea_nm = np.zeros((S, H), dtype=np.float32)
        m = slot_edge >= 0
        ea_nm[m] = edge_attr[slot_edge[m]]
        ea_fm = np.ascontiguousarray(ea_nm.T).astype(ml_dtypes.bfloat16)

        c2n = st["col2node"][c]
        xn = np.zeros((W, H), dtype=np.float32)
        real = c2n >= 0
        xn[real] = x[c2n[real]]
        xf = np.ascontiguousarray(xn.T)
        xn = xn.astype(ml_dtypes.bfloat16)

        vecs = vecs_base.copy()
        vecs[:, 22] = float(W - NOWN)  # dummy count

        in_maps.append(
            {
                "idx": idx_full,
                "ea": ea_fm,
                "xnm": xn,
                "xfm": xf,
                "w1": np.ascontiguousarray(np.asarray(W1, np.float32)),
                "w2": np.ascontiguousarray(np.asarray(W2, np.float32)),
                "vecs": vecs,
            }
        )

    return st, nc, in_maps


def unshard(st, results):
    out = np.zeros((N_NODES, H), dtype=np.float32)
    for c in range(CORES):
        o = results[c]["out"]  # [H, W]
        c2n = st["col2node"][c]
        real = c2n >= 0
        out[c2n[real]] = o.T[real]
    return out


def kernel(x, edge_index, edge_attr, W1, b1, g1, be1, W2, b2, bn_g, bn_b):
    st, nc, in_maps = prepare(
        x, edge_index, edge_attr, W1, b1, g1, be1, W2, b2, bn_g, bn_b
    )
    res = run_bass_kernel_spmd(nc, in_maps, core_ids=list(range(CORES)))
    kernel.last_results = res
    return unshard(st, res.results)


if __name__ == "__main__":
    # smoke test with random data
    rng = np.random.default_rng(0)
    x = rng.standard_normal((N_NODES, H)).astype(np.float32)
    ei = rng.integers(0, N_NODES, (2, N_EDGES)).astype(np.int64)
    ea = rng.standard_normal((N_EDGES, H)).astype(np.float32)
    s = 1.0 / np.sqrt(H)
    W1 = (rng.standard_normal((4, H, H)) * s).astype(np.float32)
    W2 = (rng.standard_normal((4, H, H)) * s).astype(np.float32)
    z = np.zeros((4, H), np.float32)
    o = np.ones((4, H), np.float32)
    out = kernel(x, ei, ea, W1, z, o, z, W2, z, np.ones((3, H), np.float32), np.zeros((3, H), np.float32))
    print("out", out.shape, out.dtype, np.abs(out).mean())



# revision 18
# speedup vs baseline: 1.7551x; 1.0018x over previous
"""GINEConv 4-layer encoder on 8 Trainium2 NeuronCores.

Strategy (graph/data parallel per sharding hint):
  - Nodes partitioned across 8 cores (6250 each, contiguous).  Each core owns
    the edges whose *dst* it owns and computes aggregation + MLP + BN for its
    own nodes only.  Full node features are exchanged via AllGather each layer
    (random graph => halo == everything).
  - Message phase: z table (bf16, node-major) in DRAM; per-edge rows fetched
    with dma_gather(transpose=True) giving feature-major [128, n_edges] tiles;
    msg = relu(gather + edge_attr) on DVE; aggregation = segmented
    tensor_reduce over host-built degree-bucketed, padded edge streams.
  - dma_gather indices are int16 (max 32768 rows), so the table is split in
    two halves (src node < 25000 vs >= 25000) and each dst's edges are
    bucketed per half: cell (B0, B1) = pow2-padded in-degree per half.
  - MLP (two 128x128 matmuls) on TensorE in f32; BatchNorm via local
    sum/sumsq (ACT accum) + tiny AllReduce; normalize+relu fused in one
    ScalarE activation (per-partition scale/bias).
  - SPMD: one program for all cores; all shapes uniform across cores (cell
    counts maxed over cores, dummy dst columns padded; stats corrected by a
    per-core dummy-count input).
"""

import sys

for p in ("/opt/trn_rl_repo",):
    if p not in sys.path:
        sys.path.insert(0, p)

import numpy as np
import ml_dtypes

import concourse.bacc as bacc
import concourse.bass as bass
import concourse.mybir as mybir
import concourse.tile as tile
from concourse.bass_utils import run_bass_kernel_spmd
from concourse.masks import make_identity

# problem constants (hardcoded per harness contract)
N_NODES = 50000
N_EDGES = 640000
H = 128
N_LAYERS = 4
BN_EPS = 1e-5
CORES = 8
NOWN = N_NODES // CORES  # 6250
HALF_NODE = N_NODES // 2  # src < 25000 -> half 0
CH = 3072  # slots per gather call / msg tile (must fit SWDGE desc ring)
USE_PREP = False  # prepare_only+trigger pipelining (False = immediate gathers)
F32 = mybir.dt.float32
BF16 = mybir.dt.bfloat16
I16 = mybir.dt.int16


def _pow2ceil(d):
    # v2: exact-degree cells — no pow2 padding. Cell = (d0, d1) pair; the
    # cross-core max still pads dummy columns, but slot count drops ~13%
    # and, more importantly, every slot is a real edge (fewer descriptors
    # for the Q7 SWDGE generator, which is the critical path).
    return np.asarray(d).copy()


class _Desc:
    __slots__ = ("chunk", "off", "nd", "b", "col", "sect")

    def __init__(self, chunk, off, nd, b, col, sect):
        self.chunk, self.off, self.nd, self.b, self.col, self.sect = (
            chunk, off, nd, b, col, sect,
        )


def _build_structure(src, dst):
    """Uniform (cross-core) stream/cell structure + per-core slot data."""
    core_of_dst = dst // NOWN
    half_of_src = (src >= HALF_NODE).astype(np.int64)

    # counts per (dst, half)
    key_dh = dst * 2 + half_of_src
    cnt = np.bincount(key_dh, minlength=2 * N_NODES)
    d0 = cnt[0::2]
    d1 = cnt[1::2]
    B0 = _pow2ceil(d0)
    B1 = _pow2ceil(d1)

    # order edges grouped by (dst, half); within groups order arbitrary
    edge_order = np.argsort(key_dh, kind="stable")
    grp_starts = np.zeros(2 * N_NODES, dtype=np.int64)
    grp_starts[1:] = np.cumsum(cnt)[:-1]

    # per-core cell membership
    cell_key = B0 * 100000 + B1  # scalar key per node
    cells_all = {}
    percore_members = []  # core -> {cellkey: np.array local dst ids}
    for c in range(CORES):
        nodes = np.arange(c * NOWN, (c + 1) * NOWN)
        k = cell_key[nodes]
        order = np.argsort(k, kind="stable")
        ks, idx_start = np.unique(k[order], return_index=True)
        members = {}
        for i, kk in enumerate(ks):
            a = idx_start[i]
            b = idx_start[i + 1] if i + 1 < len(ks) else NOWN
            members[int(kk)] = nodes[order[a:b]]  # global node ids
        percore_members.append(members)
        for kk, m in members.items():
            cells_all[kk] = max(cells_all.get(kk, 0), len(m))

    # pad target width W; guarantee >=1 dummy in cell (0,0) for every core
    w_raw = sum(cells_all.values())
    if 0 not in cells_all:
        cells_all[0] = 0
    W = ((w_raw + 1 + 127) // 128 + 0) * 128
    cells_all[0] += W - w_raw
    assert 16 + 4 * W <= 32768, f"W={W} too large for int16 half-tables"

    cell_keys = sorted(cells_all)  # (0,0) first
    col_start = {}
    pos = 0
    for kk in cell_keys:
        col_start[kk] = pos
        pos += cells_all[kk]
    assert pos == W
    vcol = col_start[0] + cells_all[0] - 1  # guaranteed dummy column

    # column -> node per core; node -> column global
    col2node = np.full((CORES, W), -1, dtype=np.int64)
    for c in range(CORES):
        for kk in cell_keys:
            m = percore_members[c].get(kk)
            if m is not None and len(m):
                s = col_start[kk]
                col2node[c, s : s + len(m)] = m
    node_col = np.zeros(N_NODES, dtype=np.int64)
    for c in range(CORES):
        real = col2node[c] >= 0
        node_col[col2node[c][real]] = np.nonzero(real)[0]

    # stream layout (shared across cores): walk cells per section.
    # v2: section tails round to 128 (gather num_idxs granularity), not CH —
    # no filler slots gathered for the tail chunk.
    descs = []
    sect_len = []
    for sect in (0, 1):
        pos = 0
        for kk in cell_keys:
            b = (kk // 100000) if sect == 0 else (kk % 100000)
            if b == 0:
                continue
            nd_left = cells_all[kk]
            col = col_start[kk]
            while nd_left:
                room = (-pos) % CH
                if room == 0:
                    room = CH
                fit = min(nd_left, room // b)
                if fit == 0:
                    pos += room  # filler to chunk boundary
                    continue
                descs.append(_Desc(pos // CH, pos % CH, fit, b, col, sect))
                pos += fit * b
                col += fit
                nd_left -= fit
        pos = ((pos + 127) // 128) * 128
        sect_len.append(pos)

    S0, S1 = sect_len
    NCH0 = (S0 + CH - 1) // CH
    NCH1 = (S1 + CH - 1) // CH
    # shift section-1 descriptors to global chunk ids
    for d in descs:
        if d.sect == 1:
            d.chunk += NCH0
    S = S0 + S1
    # global chunk table: (start offset in flat stream, size)
    chunk_start, chunk_size = [], []
    for k in range(NCH0):
        chunk_start.append(k * CH)
        chunk_size.append(min(CH, S0 - k * CH))
    for k in range(NCH1):
        chunk_start.append(S0 + k * CH)
        chunk_size.append(min(CH, S1 - k * CH))
    HALF_ROWS = 16 + 4 * W  # table rows in half 0
    R = 32 + 8 * W

    return dict(
        d0=d0, d1=d1, B0=B0, B1=B1, W=W, S0=S0, S1=S1, S=S, NCH0=NCH0,
        R=R, HALF_ROWS=HALF_ROWS, vcol=vcol, descs=descs,
        chunk_start=chunk_start, chunk_size=chunk_size,
        col2node=col2node, node_col=node_col,
        edge_order=edge_order, grp_starts=grp_starts, cnt=cnt,
        cells_all=cells_all, cell_keys=cell_keys, col_start=col_start,
    )


def _fill_core_streams(st, src, core):
    """Per-core slot arrays: table row per slot + edge id per slot."""
    W, S, CH_ = st["W"], st["S"], CH
    HALF_ROWS = st["HALF_ROWS"]
    clamp = [0, 16 + 8 * W]  # zero rows: head block (half0), tail block (half1)
    slot_row = np.full(S, -1, dtype=np.int64)
    slot_edge = np.full(S, -1, dtype=np.int64)

    node_pos = 16 + (np.arange(N_NODES) // NOWN) * W + st["node_col"]  # table row

    for d in st["descs"]:
        sect = d.sect
        base = st["chunk_start"][d.chunk] + d.off
        cols = np.arange(d.col, d.col + d.nd)
        nodes = st["col2node"][core, cols]  # may be -1 (dummy)
        # padded [nd, b] matrices
        rows = np.full((d.nd, d.b), clamp[sect], dtype=np.int64)
        eids = np.full((d.nd, d.b), -1, dtype=np.int64)
        realm = nodes >= 0
        if realm.any():
            rn = nodes[realm]
            g = rn * 2 + sect
            counts = st["cnt"][g]
            starts = st["grp_starts"][g]
            tot = counts.sum()
            if tot:
                # gather the edge ids for all real dsts in this desc
                reps = np.repeat(np.arange(len(rn)), counts)
                offs = np.arange(tot) - np.repeat(
                    np.cumsum(counts) - counts, counts
                )
                eid = st["edge_order"][
                    np.repeat(starts, counts) + offs
                ]
                rrows = np.nonzero(realm)[0][reps]
                rows[rrows, offs] = node_pos[src[eid]]
                eids[rrows, offs] = eid
        sl = slice(base, base + d.nd * d.b)
        slot_row[sl] = rows.reshape(-1)
        slot_edge[sl] = eids.reshape(-1)

    # fillers / unused slots -> clamp row of their section
    for sect, lo, hi in ((0, 0, st["S0"]), (1, st["S0"], S)):
        seg = slot_row[lo:hi]
        seg[seg < 0] = clamp[sect]
    # int16 local indices
    idx = slot_row.copy()
    idx[st["S0"]:] -= HALF_ROWS
    assert idx.min() >= 0 and idx.max() < 32768
    return slot_row, slot_edge, idx.astype(np.int16)


def _build_program(st):
    W, S, R = st["W"], st["S"], st["R"]
    HALF_ROWS, NCH0 = st["HALF_ROWS"], st["NCH0"]
    NCH = len(st["chunk_size"])
    NB = W // 128
    vcol = st["vcol"]
    NQ = 2  # SWDGE queues; alternate so desc-gen(k+1) overlaps drain(k)

    nc = bacc.Bacc(
        "TRN2", target_bir_lowering=False, debug=False, num_devices=CORES,
        num_swdge_queues=NQ,
    )

    t_idx = nc.dram_tensor("idx", [128, S // 16], I16, kind="ExternalInput")
    t_ea = nc.dram_tensor("ea", [H, S], BF16, kind="ExternalInput")
    t_xnm = nc.dram_tensor("xnm", [W, H], BF16, kind="ExternalInput")
    t_xfm = nc.dram_tensor("xfm", [H, W], F32, kind="ExternalInput")
    t_w1 = nc.dram_tensor("w1", [N_LAYERS, H, H], F32, kind="ExternalInput")
    t_w2 = nc.dram_tensor("w2", [N_LAYERS, H, H], F32, kind="ExternalInput")
    t_vecs = nc.dram_tensor("vecs", [H, 32], F32, kind="ExternalInput")
    t_out = nc.dram_tensor("out", [H, W], F32, kind="ExternalOutput")

    # vec column map (matches host packing below)
    VB1, VG1, VBE1, VB2 = 0, 4, 8, 12
    VBNG, VBNB, VNDUM = 16, 19, 22
    # scratch vec columns
    VSUM, VSQ, VASUM, VASQ = 23, 24, 25, 26
    rg = [list(range(CORES))]

    with tile.TileContext(nc) as tc:
        with (
            tc.tile_pool(name="sb", bufs=1) as sb,
            tc.tile_pool(name="db", bufs=2) as db,
            tc.tile_pool(name="ps", bufs=2, space="PSUM") as ps,
            tc.tile_pool(name="pt", bufs=2, space="PSUM") as pt,
            tc.tile_pool(name="dram", bufs=1, space="DRAM") as dr,
        ):
            idx_sb = sb.tile([128, S // 16], I16)
            z = sb.tile([128, W], F32)
            agg = sb.tile([128, W], F32)
            agg2 = sb.tile([128, W], F32)
            wt = sb.tile([128, 8 * H], F32)
            vec = sb.tile([128, 32], F32)
            sc = sb.tile([128, 16], F32)  # scratch per-partition scalars
            ident = sb.tile([128, 128], F32)
            znm = sb.tile([128, NB, 128], BF16)
            sq16 = sb.tile([128, W], BF16)
            zhead = sb.tile([16, H], BF16)

            table = dr.tile([R, H], BF16)
            agi = dr.tile([W, H], BF16)
            sti = dr.tile([128, 2], F32)
            sto = dr.tile([128, 2], F32)

            dma_sems = [nc.alloc_semaphore(f"swdge_dma{q}") for q in range(NQ)]

            make_identity(nc, ident[:])

            # ---- one-time init ----
            nc.sync.dma_start(out=idx_sb[:], in_=t_idx[:])
            nc.sync.dma_start(out=z[:], in_=t_xfm[:])
            nc.sync.dma_start(
                out=wt[:, 0 : 4 * H].rearrange("p (l o) -> p l o", l=N_LAYERS),
                in_=t_w1.rearrange("l i o -> i l o"),
            )
            nc.sync.dma_start(
                out=wt[:, 4 * H : 8 * H].rearrange("p (l o) -> p l o", l=N_LAYERS),
                in_=t_w2.rearrange("l i o -> i l o"),
            )
            nc.sync.dma_start(out=vec[:], in_=t_vecs[:])
            # zero table head/tail clamp rows
            nc.vector.memset(zhead[:], 0)
            nc.sync.dma_start(out=table[0:16, :], in_=zhead[:])
            nc.sync.dma_start(out=table[16 + 8 * W : R, :], in_=zhead[:])
            # layer-0 table: own x block (bf16 from host) and AllGather
            nc.gpsimd.dma_start(out=agi[:, :], in_=t_xnm[:, :])

            def allgather():
                nc.gpsimd.collective_compute(
                    "AllGather",
                    mybir.AluOpType.bypass,
                    replica_groups=rg,
                    ins=[agi[:, :]],
                    outs=[table[16 : 16 + 8 * W, :]],
                )

            allgather()

            descs_by_chunk = [[] for _ in range(NCH)]
            for d in st["descs"]:
                descs_by_chunk[d.chunk].append(d)

            def bn_stats_and_coefs(h_t, gcol, bcol, extra_relu):
                """stats(h) -> AllReduce -> write scale into sc[:,0], bias sc[:,1]."""
                nc.scalar.activation(
                    out=sq16[:], in_=h_t[:], func=mybir.ActivationFunctionType.Identity,
                    accum_out=vec[:, VSUM : VSUM + 1],
                )
                nc.scalar.activation(
                    out=sq16[:], in_=h_t[:], func=mybir.ActivationFunctionType.Square,
                    accum_out=vec[:, VSQ : VSQ + 1],
                )
                v = h_t[:, vcol : vcol + 1]
                # sum -= ndum*v ; sq -= ndum*v^2
                nc.vector.tensor_tensor(
                    out=sc[:, 2:3], in0=vec[:, VNDUM : VNDUM + 1], in1=v,
                    op=mybir.AluOpType.mult,
                )
                nc.vector.tensor_tensor(
                    out=vec[:, VSUM : VSUM + 1], in0=vec[:, VSUM : VSUM + 1],
                    in1=sc[:, 2:3], op=mybir.AluOpType.subtract,
                )
                nc.vector.tensor_tensor(
                    out=sc[:, 3:4], in0=v, in1=v, op=mybir.AluOpType.mult
                )
                nc.vector.tensor_tensor(
                    out=sc[:, 3:4], in0=sc[:, 3:4], in1=vec[:, VNDUM : VNDUM + 1],
                    op=mybir.AluOpType.mult,
                )
                nc.vector.tensor_tensor(
                    out=vec[:, VSQ : VSQ + 1], in0=vec[:, VSQ : VSQ + 1],
                    in1=sc[:, 3:4], op=mybir.AluOpType.subtract,
                )
                nc.sync.dma_start(out=sti[:, :], in_=vec[:, VSUM : VSUM + 2])
                nc.gpsimd.collective_compute(
                    "AllReduce", mybir.AluOpType.add, replica_groups=rg,
                    ins=[sti[:, :]], outs=[sto[:, :]],
                )
                nc.sync.dma_start(out=vec[:, VASUM : VASUM + 2], in_=sto[:, :])
                inv_n = 1.0 / float(N_NODES)
                nc.vector.tensor_scalar_mul(
                    out=sc[:, 4:5], in0=vec[:, VASUM : VASUM + 1], scalar1=inv_n
                )  # mean
                nc.vector.tensor_scalar_mul(
                    out=sc[:, 5:6], in0=vec[:, VASQ : VASQ + 1], scalar1=inv_n
                )  # E[x^2]
                nc.vector.tensor_tensor(
                    out=sc[:, 6:7], in0=sc[:, 4:5], in1=sc[:, 4:5],
                    op=mybir.AluOpType.mult,
                )
                nc.vector.tensor_tensor(
                    out=sc[:, 6:7], in0=sc[:, 5:6], in1=sc[:, 6:7],
                    op=mybir.AluOpType.subtract,
                )  # var
                nc.vector.tensor_scalar_add(
                    out=sc[:, 6:7], in0=sc[:, 6:7], scalar1=float(BN_EPS)
                )
                nc.scalar.activation(
                    out=sc[:, 7:8], in_=sc[:, 6:7],
                    func=mybir.ActivationFunctionType.Sqrt,
                )
                nc.vector.reciprocal(out=sc[:, 8:9], in_=sc[:, 7:8])  # rstd
                nc.vector.tensor_tensor(
                    out=sc[:, 0:1], in0=gcol, in1=sc[:, 8:9], op=mybir.AluOpType.mult
                )  # a = gamma * rstd
                nc.vector.tensor_tensor(
                    out=sc[:, 9:10], in0=sc[:, 4:5], in1=sc[:, 0:1],
                    op=mybir.AluOpType.mult,
                )
                nc.vector.tensor_tensor(
                    out=sc[:, 1:2], in0=bcol, in1=sc[:, 9:10],
                    op=mybir.AluOpType.subtract,
                )  # b = beta - mean*a

            # ---- layers ----
            for L in range(N_LAYERS):
                nc.vector.memset(agg[:], 0)
                nc.vector.memset(agg2[:], 0)
                for k in range(NCH):
                    cs = st["chunk_start"][k]
                    cw = st["chunk_size"][k]
                    zg = db.tile([128, cw], BF16, tag="zg")
                    eat = db.tile([128, cw], BF16, tag="ea")
                    if k < NCH0:
                        src_ap = table[0:HALF_ROWS, :]
                    else:
                        src_ap = table[HALF_ROWS:R, :]
                    q = k % NQ
                    if USE_PREP:
                        # prepare_only: the gather instruction retires at
                        # descriptor-gen completion, so gen(k+1) overlaps
                        # the DMA drain of chunk k instead of serializing.
                        nc.gpsimd.dma_gather(
                            zg[:].rearrange("p (o c) -> p o c", o=1),
                            src_ap,
                            idx_sb[:, cs // 16 : (cs + cw) // 16],
                            cw, cw, H, transpose=True, single_packet=False,
                            prepare_only=True, sem=dma_sems[q], queue_num=q,
                        )
                        nc.gpsimd.trigger_dma(count=None, queue_num=q)
                    else:
                        nc.gpsimd.dma_gather(
                            zg[:].rearrange("p (o c) -> p o c", o=1),
                            src_ap,
                            idx_sb[:, cs // 16 : (cs + cw) // 16],
                            cw, cw, H, transpose=True, single_packet=False,
                        )
                    nc.sync.dma_start(
                        out=eat[:], in_=t_ea[:, cs : cs + cw]
                    )
                    nc.vector.tensor_tensor(
                        out=zg[:], in0=zg[:], in1=eat[:],
                        op=mybir.AluOpType.add,
                    )
                    nc.vector.tensor_scalar_max(
                        out=zg[:], in0=zg[:], scalar1=0.0
                    )
                    for d in descs_by_chunk[k]:
                        tgt = agg if d.sect == 0 else agg2
                        nc.vector.tensor_reduce(
                            out=tgt[:, d.col : d.col + d.nd],
                            in_=zg[:, d.off : d.off + d.nd * d.b].rearrange(
                                "p (n b) -> p n b", b=d.b
                            ),
                            axis=mybir.AxisListType.X,
                            op=mybir.AluOpType.add,
                        )
                # h = z + agg + agg2  (into agg)
                nc.vector.tensor_tensor(
                    out=agg[:], in0=z[:], in1=agg[:], op=mybir.AluOpType.add
                )
                nc.vector.tensor_tensor(
                    out=agg[:], in0=agg[:], in1=agg2[:], op=mybir.AluOpType.add
                )
                # h1 = h @ W1[L] + b1[L] -> agg2
                for j in range(0, W, 512):
                    je = min(j + 512, W)
                    pm = ps.tile([128, 512], F32, tag="mm")
                    nc.tensor.matmul(
                        out=pm[:, : je - j],
                        lhsT=wt[:, L * H : (L + 1) * H],
                        rhs=agg[:, j:je],
                        start=True, stop=True,
                    )
                    nc.scalar.activation(
                        out=agg2[:, j:je], in_=pm[:, : je - j],
                        func=mybir.ActivationFunctionType.Identity,
                        bias=vec[:, VB1 + L : VB1 + L + 1],
                    )
                # BN1 + relu
                bn_stats_and_coefs(
                    agg2, vec[:, VG1 + L : VG1 + L + 1], vec[:, VBE1 + L : VBE1 + L + 1],
                    extra_relu=True,
                )
                nc.scalar.activation(
                    out=agg2[:], in_=agg2[:],
                    func=mybir.ActivationFunctionType.Relu,
                    bias=sc[:, 1:2], scale=sc[:, 0:1],
                )
                # z_raw = relu(h1n @ W2[L] + b2[L]) -> agg
                for j in range(0, W, 512):
                    je = min(j + 512, W)
                    pm = ps.tile([128, 512], F32, tag="mm")
                    nc.tensor.matmul(
                        out=pm[:, : je - j],
                        lhsT=wt[:, (4 + L) * H : (5 + L) * H],
                        rhs=agg2[:, j:je],
                        start=True, stop=True,
                    )
                    nc.scalar.activation(
                        out=agg[:, j:je], in_=pm[:, : je - j],
                        func=mybir.ActivationFunctionType.Relu,
                        bias=vec[:, VB2 + L : VB2 + L + 1],
                    )
                if L < N_LAYERS - 1:
                    # inter-layer BN -> new z
                    bn_stats_and_coefs(
                        agg, vec[:, VBNG + L : VBNG + L + 1],
                        vec[:, VBNB + L : VBNB + L + 1], extra_relu=False,
                    )
                    nc.scalar.activation(
                        out=z[:], in_=agg[:],
                        func=mybir.ActivationFunctionType.Identity,
                        bias=sc[:, 1:2], scale=sc[:, 0:1],
                    )
                    # table update: transpose z -> node-major bf16, AG
                    for jb in range(NB):
                        pmt = pt.tile([128, 128], F32, tag="tr")
                        nc.tensor.transpose(
                            out=pmt[:],
                            in_=z[:, jb * 128 : (jb + 1) * 128],
                            identity=ident[:],
                        )
                        nc.scalar.activation(
                            out=znm[:, jb, :], in_=pmt[:],
                            func=mybir.ActivationFunctionType.Copy,
                        )
                    nc.sync.dma_start(
                        out=agi.rearrange("(nb p) f -> p nb f", p=128),
                        in_=znm[:, :, :],
                    )
                    allgather()
                else:
                    nc.sync.dma_start(out=t_out[:, :], in_=agg[:])

    nc.compile()
    return nc


_CACHE = {}


def prepare(x, edge_index, edge_attr, W1, b1, g1, be1, W2, b2, bn_g, bn_b):
    """Host prep + program build; returns (st, nc, in_maps)."""
    x = np.asarray(x, dtype=np.float32)
    edge_index = np.asarray(edge_index, dtype=np.int64)
    edge_attr = np.asarray(edge_attr, dtype=np.float32)
    src = edge_index[0].astype(np.int64)
    dst = edge_index[1].astype(np.int64)

    ck = hash(edge_index.tobytes())
    if ck in _CACHE:
        st, nc, percore = _CACHE[ck]
    else:
        st = _build_structure(src, dst)
        percore = [
            _fill_core_streams(st, src, c) for c in range(CORES)
        ]
        nc = _build_program(st)
        _CACHE[ck] = (st, nc, percore)

    W, S = st["W"], st["S"]
    # weight vec packing [128, 32]
    vecs_base = np.zeros((H, 32), dtype=np.float32)
    vecs_base[:, 0:4] = np.asarray(b1, np.float32).T
    vecs_base[:, 4:8] = np.asarray(g1, np.float32).T
    vecs_base[:, 8:12] = np.asarray(be1, np.float32).T
    vecs_base[:, 12:16] = np.asarray(b2, np.float32).T
    vecs_base[:, 16:19] = np.asarray(bn_g, np.float32).T
    vecs_base[:, 19:22] = np.asarray(bn_b, np.float32).T

    in_maps = []
    for c in range(CORES):
        slot_row, slot_edge, idx16 = percore[c]
        wrapped = np.ascontiguousarray(idx16.reshape(-1, 16).T)  # [16, S/16]
        idx_full = np.ascontiguousarray(np.tile(wrapped, (8, 1)))  # [128, S/16]

        ea_nm = np.zeros((S, H), dtype=np.float32)
        m = slot_edge >= 0
        ea_nm[m] = edge_attr[slot_edge[m]]
        ea_fm = np.ascontiguousarray(ea_nm.T).astype(ml_dtypes.bfloat16)

        c2n = st["col2node"][c]
        xn = np.zeros((W, H), dtype=np.float32)
        real = c2n >= 0
        xn[real] = x[c2n[real]]
        xf = np.ascontiguousarray(xn.T)
        xn = xn.astype(ml_dtypes.bfloat16)

        vecs = vecs_base.copy()
        vecs[:, 22] = float(W - NOWN)  # dummy count

        in_maps.append(
            {
                "idx": idx_full,
                "ea": ea_fm,
                "xnm": xn,
                "xfm": xf,
                "w1": np.ascontiguousarray(np.asarray(W1, np.float32)),
                "w2": np.ascontiguousarray(np.asarray(W2, np.float32)),
                "vecs": vecs,
            }
        )

    return st, nc, in_maps


def unshard(st, results):
    out = np.zeros((N_NODES, H), dtype=np.float32)
    for c in range(CORES):
        o = results[c]["out"]  # [H, W]
        c2n = st["col2node"][c]
        real = c2n >= 0
        out[c2n[real]] = o.T[real]
    return out


def kernel(x, edge_index, edge_attr, W1, b1, g1, be1, W2, b2, bn_g, bn_b):
    st, nc, in_maps = prepare(
        x, edge_index, edge_attr, W1, b1, g1, be1, W2, b2, bn_g, bn_b
    )
    res = run_bass_kernel_spmd(nc, in_maps, core_ids=list(range(CORES)))
    kernel.last_results = res
    return unshard(st, res.results)


if __name__ == "__main__":
    # smoke test with random data
    rng = np.random.default_rng(0)
    x = rng.standard_normal((N_NODES, H)).astype(np.float32)
    ei = rng.integers(0, N_NODES, (2, N_EDGES)).astype(np.int64)
    ea = rng.standard_normal((N_EDGES, H)).astype(np.float32)
    s = 1.0 / np.sqrt(H)
    W1 = (rng.standard_normal((4, H, H)) * s).astype(np.float32)
    W2 = (rng.standard_normal((4, H, H)) * s).astype(np.float32)
    z = np.zeros((4, H), np.float32)
    o = np.ones((4, H), np.float32)
    out = kernel(x, ei, ea, W1, z, o, z, W2, z, np.ones((3, H), np.float32), np.zeros((3, H), np.float32))
    print("out", out.shape, out.dtype, np.abs(out).mean())

